# revision 1
# baseline (speedup 1.0000x reference)
"""Llama MHA (B=2, S=2048, D=2048, H=16, causal, RoPE) on 8 trn2 cores.

Sharding: data-parallel over batch (2 groups of 4 cores) x tensor-parallel
over heads (4 heads per core). Each core computes, for its (batch, 4 heads):
  qT/kT = w^T x^T  (features on partitions, seq on free dim)
  RoPE on qT/kT (weights column-permuted on host so even/odd feature pairs
  land de-interleaved: rows 0:64 = even, 64:128 = odd; dot products are
  permutation-invariant so scores match the reference exactly)
  scoresT[k,q] blocks -> exp (no max subtraction needed: |score*scale| <~ 6)
  -> causal mask on diagonal blocks -> PV matmuls + all-ones-matrix
  denominator matmuls (every PSUM row = key-sum, so the broadcast for the
  normalization divide is free)
  -> normalize -> out projection partial resT = wo^T attnT.
Host sums the 4 partials per batch and transposes back.

All matmuls in bf16 (fp32 PSUM accumulation); softmax/normalization in fp32.
"""

import numpy as np
import ml_dtypes

import concourse.bass as bass
import concourse.mybir as mybir
import concourse.tile as tile
from concourse import bacc
from concourse.bass_utils import run_bass_kernel_spmd

B, S, D, H = 2, 2048, 2048, 16
DH = D // H            # 128 head dim
HPC = 4                # heads per core
N_CORES = 8
FH = HPC * DH          # 512 features per core
P = 128
KT = D // P            # 16 k-tiles over D
SC = S // 512          # 4 seq chunks of 512
ST = S // P            # 16 seq blocks of 128
THETA = 10000.0
SCALE = 1.0 / np.sqrt(DH)

DT = mybir.dt.bfloat16
NPDT = ml_dtypes.bfloat16

_prog_cache = {}


def _build():
    if "nc" in _prog_cache:
        return _prog_cache["nc"]
    nc = bacc.Bacc(None, target_bir_lowering=False, debug=False)

    xT = nc.dram_tensor("xT", [D, S], DT, kind="ExternalInput")
    wq = nc.dram_tensor("wq", [D, FH], DT, kind="ExternalInput")
    wk = nc.dram_tensor("wk", [D, FH], DT, kind="ExternalInput")
    wv = nc.dram_tensor("wv", [D, FH], DT, kind="ExternalInput")
    wo = nc.dram_tensor("wo", [FH, D], DT, kind="ExternalInput")
    cc = nc.dram_tensor("cc", [P, S], mybir.dt.float32, kind="ExternalInput")
    ss = nc.dram_tensor("ss", [P, S], mybir.dt.float32, kind="ExternalInput")
    masks = nc.dram_tensor("masks", [P, 4, 512], DT, kind="ExternalInput")
    resT = nc.dram_tensor("resT", [D, S], mybir.dt.float32, kind="ExternalOutput")

    f32 = mybir.dt.float32

    with tile.TileContext(nc) as tc:
        with (
            tc.tile_pool(name="persist", bufs=1) as pp,
            tc.tile_pool(name="psA", bufs=4, space="PSUM") as psA,
            tc.tile_pool(name="psO", bufs=2, space="PSUM") as psO,
            tc.tile_pool(name="psD", bufs=2, space="PSUM") as psD,
        ):
            qT = pp.tile([P, HPC, S], DT)     # per head: rows=feat, free=seq
            kT = pp.tile([P, HPC, S], DT)
            vn = pp.tile([P, ST, FH], DT)     # v natural: [seq-block, feat]
            attnT = pp.tile([P, HPC, S], DT)  # normalized attention output^T
            cc_t = pp.tile([P, S], f32)
            ss_t = pp.tile([P, S], f32)
            mask_t = pp.tile([P, 4, 512], DT)
            ones_mat = pp.tile([P, P], DT)    # denominator stationary: the
                                              # [128,128] all-ones matrix makes
                                              # every PSUM row the key-sum, so
                                              # the broadcast is free

            nc.vector.memset(ones_mat, 1.0)
            wo_t = pp.tile([P, HPC, D], DT)

            # ---------------- Phase 1: projections + RoPE -----------------
            with (
                tc.tile_pool(name="wpool", bufs=1) as wp,
                tc.tile_pool(name="xpool", bufs=2) as xp,
                tc.tile_pool(name="ropetmp", bufs=4) as rp,
            ):
                wq_t = wp.tile([P, KT, FH], DT)
                wk_t = wp.tile([P, KT, FH], DT)
                wv_t = wp.tile([P, KT, FH], DT)
                # DMA issue order is the Sync-queue order: interleave the
                # first x chunk with wq so the first matmul chain starts as
                # early as possible; defer everything not needed immediately.
                xc0 = xp.tile([P, KT, 512], DT, tag="xc", name="xc0")
                # first pieces are half-size so the first matmul starts sooner
                for gs in (slice(0, 2), slice(2, 4), slice(4, 8),
                           slice(8, 12), slice(12, 16)):
                    nc.sync.dma_start(
                        out=wq_t[:, gs, :],
                        in_=wq.rearrange("(kt p) f -> p kt f", p=P)[:, gs, :])
                    nc.sync.dma_start(
                        out=xc0[:, gs, :],
                        in_=xT.rearrange("(kt p) s -> p kt s", p=P)[:, gs, 0:512])
                nc.sync.dma_start(out=cc_t[:, 0:512], in_=cc[:, 0:512])
                nc.sync.dma_start(out=ss_t[:, 0:512], in_=ss[:, 0:512])
                for g in range(4):
                    gs = slice(g * 4, (g + 1) * 4)
                    nc.sync.dma_start(
                        out=wk_t[:, gs, :],
                        in_=wk.rearrange("(kt p) f -> p kt f", p=P)[:, gs, :])
                nc.sync.dma_start(out=wv_t, in_=wv.rearrange("(kt p) f -> p kt f", p=P))
                nc.sync.dma_start(out=mask_t, in_=masks[:, :, :])
                nc.sync.dma_start(out=cc_t[:, 512:], in_=cc[:, 512:])
                nc.sync.dma_start(out=ss_t[:, 512:], in_=ss[:, 512:])

                for sc in range(SC):
                    if sc == 0:
                        xc = xc0
                    else:
                        xc = xp.tile([P, KT, 512], DT, tag="xc", name=f"xc{sc}")
                        for g in range(4):
                            gs = slice(g * 4, (g + 1) * 4)
                            nc.sync.dma_start(
                                out=xc[:, gs, :],
                                in_=xT.rearrange("(kt p) s -> p kt s", p=P)[
                                    :, gs, sc * 512:(sc + 1) * 512],
                            )
                    if sc == 1:
                        nc.sync.dma_start(
                            out=wo_t, in_=wo.rearrange("(ft p) d -> p ft d", p=P))
                    csl = slice(sc * 512, (sc + 1) * 512)
                    # q/k projections with RoPE fused into the PSUM drain.
                    # All q chains before k chains: the first four only need
                    # wq+xc, giving the wk DMA ~14us of PE work as cover.
                    for wt, dst in ((wq_t, qT), (wk_t, kT)):
                        for h in range(HPC):
                            fsl = slice(h * DH, (h + 1) * DH)
                            pq = psA.tile([P, 512], f32, tag="ps", name=f"pq{sc}{h}")
                            for k in range(KT):
                                nc.tensor.matmul(
                                    pq, wt[:, k, fsl], xc[:, k, :],
                                    start=(k == 0), stop=(k == KT - 1),
                                )
                            # RoPE: dst = pq*cc + swap(pq)*(+/-ss)
                            # ss_t rows 0:64 = +sin (feeds bottom), rows
                            # 64:128 = -sin (feeds top); swap is done by
                            # writing each product into the opposite half
                            # so every DVE op has aligned base partitions.
                            ta = rp.tile([P, 512], f32, tag="ta")
                            tb = rp.tile([P, 512], f32, tag="tb")
                            nc.vector.tensor_mul(ta, pq, cc_t[:, csl])
                            nc.vector.tensor_mul(
                                tb[0:64, :], pq[64:128, :], ss_t[64:128, csl])
                            nc.vector.tensor_mul(
                                tb[64:128, :], pq[0:64, :], ss_t[0:64, csl])
                            nc.vector.tensor_add(dst[:, h, csl], ta, tb)
                    # v projection straight into natural layout
                    for st4 in range(4):
                        sb = sc * 4 + st4
                        pv = psA.tile([P, FH], f32, tag="ps", name=f"pv{sc}{st4}")
                        for k in range(KT):
                            nc.tensor.matmul(
                                pv, xc[:, k, st4 * P:(st4 + 1) * P], wv_t[:, k, :],
                                start=(k == 0), stop=(k == KT - 1),
                            )
                        nc.vector.tensor_copy(vn[:, sb, :], pv)

            # ---------------- Phase 2: attention ------------------------
            with (
                tc.tile_pool(name="ppool", bufs=8) as ptp,
                tc.tile_pool(name="npool", bufs=4) as np_,
            ):
                for qc in range(SC):
                    qsl = slice(qc * 512, (qc + 1) * 512)
                    for h in range(HPC):
                        fsl = slice(h * DH, (h + 1) * DH)
                        po = psO.tile([P, 512], f32, tag="po", name=f"po{h}{qc}")
                        pd = psD.tile([P, 512], f32, tag="pd", name=f"pd{h}{qc}")
                        nkb = 4 * qc + 4
                        for kb in range(nkb):
                            ps = psA.tile([P, 512], f32, tag="ps",
                                          name=f"ps{h}{qc}{kb}")
                            nc.tensor.matmul(
                                ps, kT[:, h, kb * P:(kb + 1) * P], qT[:, h, qsl],
                                start=True, stop=True,
                            )
                            pt = ptp.tile([P, 512], DT, tag="pt")
                            nc.scalar.activation(
                                pt, ps, mybir.ActivationFunctionType.Exp,
                                scale=float(SCALE),
                            )
                            if kb >= 4 * qc:
                                nc.vector.tensor_mul(
                                    pt, pt, mask_t[:, kb - 4 * qc, :])
                            nc.tensor.matmul(
                                po, vn[:, kb, fsl], pt,
                                start=(kb == 0), stop=(kb == nkb - 1),
                            )
                            nc.tensor.matmul(
                                pd, ones_mat, pt,
                                start=(kb == 0), stop=(kb == nkb - 1),
                            )
                        bc = np_.tile([P, 512], f32, tag="bc")
                        nc.vector.reciprocal_approx_fast(out=bc, in_=pd)
                        nc.vector.tensor_mul(attnT[:, h, qsl], po, bc)

            # ---------------- Phase 3: output projection ----------------
            with (
                tc.tile_pool(name="rpool", bufs=4) as rop,
            ):
                for db in range(KT):
                    rt = rop.tile([P, S], f32, tag="rt")
                    last = db == KT - 1
                    for sc in range(SC):
                        csl = slice(sc * 512, (sc + 1) * 512)
                        pr = psA.tile([P, 512], f32, tag="ps", name=f"pr{sc}{db}")
                        for ft in range(HPC):
                            nc.tensor.matmul(
                                pr, wo_t[:, ft, db * P:(db + 1) * P],
                                attnT[:, ft, csl],
                                start=(ft == 0), stop=(ft == HPC - 1),
                            )
                        nc.vector.tensor_copy(rt[:, csl], pr)
                        if last:
                            nc.sync.dma_start(
                                out=resT[db * P:(db + 1) * P, csl],
                                in_=rt[:, csl])
                    if not last:
                        nc.sync.dma_start(
                            out=resT[db * P:(db + 1) * P, :], in_=rt)

    nc.finalize()
    _prog_cache["nc"] = nc
    return nc


def _host_inputs(x, w_q, w_k, w_v, w_o):
    """Build the 8 per-core input maps."""
    # RoPE de-interleave permutation per head: evens then odds
    i = np.arange(DH)
    perm_head = np.concatenate([i[0::2], i[1::2]])  # within-head column order

    t = np.arange(S, dtype=np.float64)
    inv_freq = 1.0 / (THETA ** (np.arange(0, DH, 2, dtype=np.float64) / DH))
    ang = np.outer(t, inv_freq)          # [S, 64]
    cosT = np.cos(ang).T.astype(np.float32)   # [64, S]
    sinT = np.sin(ang).T.astype(np.float32)
    cc = np.vstack([cosT, cosT])         # [128, S]
    ss = np.vstack([sinT, -sinT])        # +sin feeds bottom half, -sin top

    # diagonal causal masks: mask[j][k, q] = 1 if 128*j + k <= q
    kk = np.arange(P)[:, None]
    qq = np.arange(512)[None, :]
    masks = np.stack(
        [(P * j + kk <= qq) for j in range(4)], axis=1
    ).astype(NPDT)                        # [128, 4, 512]

    in_maps = []
    for core in range(N_CORES):
        b = core // 4
        h0 = (core % 4) * HPC
        cols = np.concatenate(
            [h * DH + perm_head for h in range(h0, h0 + HPC)])   # rope-permuted
        vcols = np.arange(h0 * DH, (h0 + HPC) * DH)              # natural
        in_maps.append({
            "xT": np.ascontiguousarray(x[b].T).astype(NPDT),
            "wq": np.ascontiguousarray(w_q[:, cols]).astype(NPDT),
            "wk": np.ascontiguousarray(w_k[:, cols]).astype(NPDT),
            "wv": np.ascontiguousarray(w_v[:, vcols]).astype(NPDT),
            "wo": np.ascontiguousarray(w_o[vcols, :]).astype(NPDT),
            "cc": cc,
            "ss": ss,
            "masks": masks,
        })
    return in_maps


def kernel(x, w_q, w_k, w_v, w_o, _trace=False, _results_out=None):
    x = np.asarray(x, dtype=np.float32)
    w_q = np.asarray(w_q, dtype=np.float32)
    w_k = np.asarray(w_k, dtype=np.float32)
    w_v = np.asarray(w_v, dtype=np.float32)
    w_o = np.asarray(w_o, dtype=np.float32)
    nc = _build()
    in_maps = _host_inputs(x, w_q, w_k, w_v, w_o)
    res = run_bass_kernel_spmd(
        nc, in_maps, core_ids=list(range(N_CORES)), trace=_trace)
    if _results_out is not None:
        _results_out.append(res)
    out = np.empty((B, S, D), np.float32)
    for b in range(B):
        acc = res.results[4 * b]["resT"].astype(np.float32)
        for g in range(1, 4):
            acc = acc + res.results[4 * b + g]["resT"]
        out[b] = acc.T
    return out



# revision 2
# speedup vs baseline: 1.2155x; 1.2155x over previous
"""Llama MHA (B=2, S=2048, D=2048, H=16, causal, RoPE) on 8 trn2 cores.

Sharding: data-parallel over batch (2 groups of 4 cores) x tensor-parallel
over heads (4 heads per core). Each core computes, for its (batch, 4 heads):
  qT/kT = w^T x^T  (features on partitions, seq on free dim)
  RoPE on qT/kT (weights column-permuted on host so even/odd feature pairs
  land de-interleaved; dot products are permutation-invariant so scores
  match the reference exactly). RoPE runs as one Scalar-engine PSUM drain
  to fp16 followed by all-fp16 SBUF DVE ops (4x DVE mode).
  scoresT[k,q] blocks -> exp(fp16). Causal handling: diagonal-superblock
  matmuls are narrowed to the valid query range (saves ~10% of attention
  PE cycles) and a single shared [128,128] triangular mask covers the
  128-wide diagonal window of each key block.
  Softmax denominator: pt blocks are accumulated per (head, qchunk) with
  cheap fp16 DVE adds, then ONE [128,128]-ones matmul per (head, qchunk)
  does the partition reduction with a free PSUM row-broadcast (replaces
  the per-block ones-matmuls: ~82K -> ~8K PE cycles).
  Out-projection is interleaved per query chunk so the PE keeps working
  through the activation-bound attention stretches; partial results ship
  as fp16 and the host sums the 4 partials per batch in fp32.

All matmuls in fp16 (fp32 PSUM accumulation); softmax/normalization fp32.
"""

import numpy as np

import concourse.bass as bass
import concourse.mybir as mybir
import concourse.tile as tile
from concourse import bacc
from concourse.bass_utils import run_bass_kernel_spmd

B, S, D, H = 2, 2048, 2048, 16
DH = D // H            # 128 head dim
HPC = 4                # heads per core
N_CORES = 8
FH = HPC * DH          # 512 features per core
P = 128
KT = D // P            # 16 k-tiles over D
SC = S // 512          # 4 seq chunks of 512
ST = S // P            # 16 seq blocks of 128
THETA = 10000.0
SCALE = 1.0 / np.sqrt(DH)

DT = mybir.dt.float16
NPDT = np.float16

_prog_cache = {}


def _build():
    if "nc" in _prog_cache:
        return _prog_cache["nc"]
    nc = bacc.Bacc(None, target_bir_lowering=False, debug=False)

    xT = nc.dram_tensor("xT", [D, S], DT, kind="ExternalInput")
    wq = nc.dram_tensor("wq", [D, FH], DT, kind="ExternalInput")
    wk = nc.dram_tensor("wk", [D, FH], DT, kind="ExternalInput")
    wv = nc.dram_tensor("wv", [D, FH], DT, kind="ExternalInput")
    wo = nc.dram_tensor("wo", [FH, D], DT, kind="ExternalInput")
    cc = nc.dram_tensor("cc", [P, S], DT, kind="ExternalInput")
    ss = nc.dram_tensor("ss", [P, S], DT, kind="ExternalInput")
    masks = nc.dram_tensor("masks", [P, P], DT, kind="ExternalInput")
    resT = nc.dram_tensor("resT", [D, S], DT, kind="ExternalOutput")

    f32 = mybir.dt.float32
    Exp = mybir.ActivationFunctionType.Exp
    Copy = mybir.ActivationFunctionType.Copy

    with tile.TileContext(nc) as tc:
        with (
            tc.tile_pool(name="persist", bufs=1) as pp,
            tc.tile_pool(name="psA", bufs=4, space="PSUM") as psA,
            tc.tile_pool(name="psO", bufs=2, space="PSUM") as psO,
            tc.tile_pool(name="psD", bufs=2, space="PSUM") as psD,
        ):
            qT = pp.tile([P, HPC, S], DT)     # per head: rows=feat, free=seq
            kT = pp.tile([P, HPC, S], DT)
            vn = pp.tile([P, ST, FH], DT)     # v natural: [seq-block, feat]
            attnT = pp.tile([P, HPC, S], DT)  # normalized attention output^T
            cc_t = pp.tile([P, S], DT)
            ss_t = pp.tile([P, S], DT)
            mask_t = pp.tile([P, P], DT)      # tri mask: 1 if k <= q (local)
            ones_mat = pp.tile([P, P], DT)    # denominator reduce stationary
            wo_t = pp.tile([P, HPC, D], DT)

            nc.vector.memset(ones_mat, 1.0)

            # ---------------- Phase 1: projections + RoPE -----------------
            with (
                tc.tile_pool(name="wpool", bufs=1) as wp,
                tc.tile_pool(name="xpool", bufs=2) as xp,
                tc.tile_pool(name="ropetmp", bufs=6) as rp,
            ):
                wq_t = wp.tile([P, KT, FH], DT)
                wk_t = wp.tile([P, KT, FH], DT)
                wv_t = wp.tile([P, KT, FH], DT)
                # DMA issue order is the Sync-queue order: interleave the
                # first x chunk with wq so the first matmul chain starts as
                # early as possible; defer everything not needed immediately.
                xc0 = xp.tile([P, KT, 512], DT, tag="xc", name="xc0")
                for gs in (slice(0, 2), slice(2, 4), slice(4, 8),
                           slice(8, 12), slice(12, 16)):
                    nc.sync.dma_start(
                        out=wq_t[:, gs, :],
                        in_=wq.rearrange("(kt p) f -> p kt f", p=P)[:, gs, :])
                    nc.sync.dma_start(
                        out=xc0[:, gs, :],
                        in_=xT.rearrange("(kt p) s -> p kt s", p=P)[:, gs, 0:512])
                nc.sync.dma_start(out=cc_t[:, 0:512], in_=cc[:, 0:512])
                nc.sync.dma_start(out=ss_t[:, 0:512], in_=ss[:, 0:512])
                for g in range(4):
                    gs = slice(g * 4, (g + 1) * 4)
                    nc.sync.dma_start(
                        out=wk_t[:, gs, :],
                        in_=wk.rearrange("(kt p) f -> p kt f", p=P)[:, gs, :])
                nc.sync.dma_start(out=wv_t, in_=wv.rearrange("(kt p) f -> p kt f", p=P))
                nc.sync.dma_start(out=mask_t, in_=masks[:, :])
                nc.sync.dma_start(out=cc_t[:, 512:], in_=cc[:, 512:])
                nc.sync.dma_start(out=ss_t[:, 512:], in_=ss[:, 512:])

                for sc in range(SC):
                    if sc == 0:
                        xc = xc0
                    else:
                        xc = xp.tile([P, KT, 512], DT, tag="xc", name=f"xc{sc}")
                        for g in range(4):
                            gs = slice(g * 4, (g + 1) * 4)
                            nc.sync.dma_start(
                                out=xc[:, gs, :],
                                in_=xT.rearrange("(kt p) s -> p kt s", p=P)[
                                    :, gs, sc * 512:(sc + 1) * 512],
                            )
                    if sc == 1:
                        nc.sync.dma_start(
                            out=wo_t, in_=wo.rearrange("(ft p) d -> p ft d", p=P))
                    csl = slice(sc * 512, (sc + 1) * 512)
                    # q/k projections. RoPE fused into the PSUM drain:
                    # Scalar engine copies PSUM->fp16 SBUF, then three
                    # all-fp16 DVE ops (eligible for the fast DVE mode).
                    # ss_t rows 0:64 = +sin, rows 64:128 = -sin; the swap is
                    # done by writing each product into the opposite half.
                    for wt, dst in ((wq_t, qT), (wk_t, kT)):
                        for h in range(HPC):
                            fsl = slice(h * DH, (h + 1) * DH)
                            pq = psA.tile([P, 512], f32, tag="ps", name=f"pq{sc}{h}")
                            for k in range(KT):
                                nc.tensor.matmul(
                                    pq, wt[:, k, fsl], xc[:, k, :],
                                    start=(k == 0), stop=(k == KT - 1),
                                )
                            pqs = rp.tile([P, 512], DT, tag="pqs")
                            ta = rp.tile([P, 512], DT, tag="ta")
                            tb = rp.tile([P, 512], DT, tag="tb")
                            nc.scalar.activation(pqs, pq, Copy)
                            nc.vector.tensor_mul(ta, pqs, cc_t[:, csl])
                            nc.vector.tensor_mul(
                                tb[0:64, :], pqs[64:128, :], ss_t[64:128, csl])
                            nc.vector.tensor_mul(
                                tb[64:128, :], pqs[0:64, :], ss_t[0:64, csl])
                            nc.vector.tensor_add(dst[:, h, csl], ta, tb)
                    # v projection straight into natural layout
                    for st4 in range(4):
                        sb = sc * 4 + st4
                        pv = psA.tile([P, FH], f32, tag="ps", name=f"pv{sc}{st4}")
                        for k in range(KT):
                            nc.tensor.matmul(
                                pv, xc[:, k, st4 * P:(st4 + 1) * P], wv_t[:, k, :],
                                start=(k == 0), stop=(k == KT - 1),
                            )
                        nc.vector.tensor_copy(vn[:, sb, :], pv)

            # ---------- Phase 2+3: attention + interleaved out-proj -------
            with (
                tc.tile_pool(name="ppool", bufs=8) as ptp,
                tc.tile_pool(name="npool", bufs=4) as np_,
                tc.tile_pool(name="dpool", bufs=3) as dp,
                tc.tile_pool(name="rpool", bufs=4) as rop,
            ):
                for qc in range(SC):
                    qsl = slice(qc * 512, (qc + 1) * 512)
                    for h in range(HPC):
                        fsl = slice(h * DH, (h + 1) * DH)
                        po = psO.tile([P, 512], f32, tag="po", name=f"po{h}{qc}")
                        pda = dp.tile([P, 512], DT, tag="pda", name=f"pda{h}{qc}")
                        nkb = 4 * qc + 4
                        for kb in range(nkb):
                            jl = kb - 4 * qc       # >=0 only on the diagonal
                            off = 128 * jl if jl > 0 else 0
                            wsl = slice(off, 512)
                            ps = psA.tile([P, 512], f32, tag="ps",
                                          name=f"ps{h}{qc}{kb}")
                            nc.tensor.matmul(
                                ps[:, wsl], kT[:, h, kb * P:(kb + 1) * P],
                                qT[:, h, qc * 512 + off:(qc + 1) * 512],
                                start=True, stop=True,
                            )
                            pt = ptp.tile([P, 512], DT, tag="pt")
                            nc.scalar.activation(
                                pt[:, wsl], ps[:, wsl], Exp, scale=float(SCALE))
                            if jl >= 0:
                                # triangular mask on the 128-wide diag window
                                nc.vector.tensor_mul(
                                    pt[:, off:off + P], pt[:, off:off + P],
                                    mask_t)
                            nc.tensor.matmul(
                                po[:, wsl], vn[:, kb, fsl], pt[:, wsl],
                                start=(kb == 0), stop=(kb == nkb - 1),
                                skip_group_check=True,
                            )
                            if kb == 0:
                                nc.vector.tensor_copy(pda, pt)
                            else:
                                nc.vector.tensor_add(
                                    pda[:, wsl], pda[:, wsl], pt[:, wsl])
                        pd = psD.tile([P, 512], f32, tag="pd", name=f"pd{h}{qc}")
                        nc.tensor.matmul(pd, ones_mat, pda, start=True, stop=True)
                        bc = np_.tile([P, 512], f32, tag="bc")
                        nc.vector.reciprocal_approx_fast(out=bc, in_=pd)
                        nc.vector.tensor_mul(attnT[:, h, qsl], po, bc)
                    # out-projection for this query chunk
                    for db in range(KT):
                        pr = psA.tile([P, 512], f32, tag="ps", name=f"pr{qc}{db}")
                        for ft in range(HPC):
                            nc.tensor.matmul(
                                pr, wo_t[:, ft, db * P:(db + 1) * P],
                                attnT[:, ft, qsl],
                                start=(ft == 0), stop=(ft == HPC - 1),
                            )
                        rt = rop.tile([P, 512], DT, tag="rt")
                        if db % 2 == 0:
                            nc.vector.tensor_copy(rt, pr)
                        else:
                            nc.scalar.activation(rt, pr, Copy)
                        nc.sync.dma_start(
                            out=resT[db * P:(db + 1) * P, qsl], in_=rt)

    nc.finalize()
    _prog_cache["nc"] = nc
    return nc


def _host_inputs(x, w_q, w_k, w_v, w_o):
    """Build the 8 per-core input maps."""
    # RoPE de-interleave permutation per head: evens then odds
    i = np.arange(DH)
    perm_head = np.concatenate([i[0::2], i[1::2]])  # within-head column order

    t = np.arange(S, dtype=np.float64)
    inv_freq = 1.0 / (THETA ** (np.arange(0, DH, 2, dtype=np.float64) / DH))
    ang = np.outer(t, inv_freq)          # [S, 64]
    cosT = np.cos(ang).T.astype(np.float32)   # [64, S]
    sinT = np.sin(ang).T.astype(np.float32)
    cc = np.vstack([cosT, cosT]).astype(NPDT)   # [128, S]
    ss = np.vstack([sinT, -sinT]).astype(NPDT)  # +sin feeds bottom half

    # shared diagonal mask: mask[k, q] = 1 if k <= q (128-wide local window)
    kk = np.arange(P)[:, None]
    qq = np.arange(P)[None, :]
    masks = (kk <= qq).astype(NPDT)      # [128, 128]

    in_maps = []
    for core in range(N_CORES):
        b = core // 4
        h0 = (core % 4) * HPC
        cols = np.concatenate(
            [h * DH + perm_head for h in range(h0, h0 + HPC)])   # rope-permuted
        vcols = np.arange(h0 * DH, (h0 + HPC) * DH)              # natural
        in_maps.append({
            "xT": np.ascontiguousarray(x[b].T).astype(NPDT),
            "wq": np.ascontiguousarray(w_q[:, cols]).astype(NPDT),
            "wk": np.ascontiguousarray(w_k[:, cols]).astype(NPDT),
            "wv": np.ascontiguousarray(w_v[:, vcols]).astype(NPDT),
            "wo": np.ascontiguousarray(w_o[vcols, :]).astype(NPDT),
            "cc": cc,
            "ss": ss,
            "masks": masks,
        })
    return in_maps


def kernel(x, w_q, w_k, w_v, w_o, _trace=False, _results_out=None):
    x = np.asarray(x, dtype=np.float32)
    w_q = np.asarray(w_q, dtype=np.float32)
    w_k = np.asarray(w_k, dtype=np.float32)
    w_v = np.asarray(w_v, dtype=np.float32)
    w_o = np.asarray(w_o, dtype=np.float32)
    nc = _build()
    in_maps = _host_inputs(x, w_q, w_k, w_v, w_o)
    res = run_bass_kernel_spmd(
        nc, in_maps, core_ids=list(range(N_CORES)), trace=_trace)
    if _results_out is not None:
        _results_out.append(res)
    out = np.empty((B, S, D), np.float32)
    for b in range(B):
        acc = res.results[4 * b]["resT"].astype(np.float32)
        for g in range(1, 4):
            acc = acc + res.results[4 * b + g]["resT"].astype(np.float32)
        out[b] = acc.T
    return out


# revision 8
# speedup vs baseline: 1.2283x; 1.0105x over previous
"""Llama MHA (B=2, S=2048, D=2048, H=16, causal, RoPE) on 8 trn2 cores.

Sharding: data-parallel over batch (2 groups of 4 cores) x tensor-parallel
over heads (4 heads per core). Each core computes, for its (batch, 4 heads):
  qT/kT = w^T x^T  (features on partitions, seq on free dim)
  RoPE on qT/kT (weights column-permuted on host so even/odd feature pairs
  land de-interleaved; dot products are permutation-invariant so scores
  match the reference exactly). RoPE runs as one Scalar-engine PSUM drain
  to fp16 followed by all-fp16 SBUF DVE ops (4x DVE mode).
  scoresT[k,q] blocks -> exp(fp16). Causal handling: diagonal-superblock
  matmuls are narrowed to the valid query range (saves ~10% of attention
  PE cycles) and a single shared [128,128] triangular mask covers the
  128-wide diagonal window of each key block.
  Softmax denominator: pt blocks are accumulated per (head, qchunk) with
  cheap fp16 DVE adds, then ONE [128,128]-ones matmul per (head, qchunk)
  does the partition reduction with a free PSUM row-broadcast (replaces
  the per-block ones-matmuls: ~82K -> ~8K PE cycles).
  Out-projection is interleaved per query chunk so the PE keeps working
  through the activation-bound attention stretches; partial results ship
  as fp16 and the host sums the 4 partials per batch in fp32.

All matmuls in fp16 (fp32 PSUM accumulation); softmax/normalization fp32.
"""

import numpy as np

import concourse.bass as bass
import concourse.mybir as mybir
import concourse.tile as tile
from concourse import bacc
from concourse.bass_utils import run_bass_kernel_spmd

B, S, D, H = 2, 2048, 2048, 16
DH = D // H            # 128 head dim
HPC = 4                # heads per core
N_CORES = 8
FH = HPC * DH          # 512 features per core
P = 128
KT = D // P            # 16 k-tiles over D
SC = S // 512          # 4 seq chunks of 512
ST = S // P            # 16 seq blocks of 128
THETA = 10000.0
SCALE = 1.0 / np.sqrt(DH)

DT = mybir.dt.float16
NPDT = np.float16

_prog_cache = {}


def _build():
    if "nc" in _prog_cache:
        return _prog_cache["nc"]
    nc = bacc.Bacc(None, target_bir_lowering=False, debug=False)

    xT = nc.dram_tensor("xT", [D, S], DT, kind="ExternalInput")
    wq = nc.dram_tensor("wq", [D, FH], DT, kind="ExternalInput")
    wk = nc.dram_tensor("wk", [D, FH], DT, kind="ExternalInput")
    wv = nc.dram_tensor("wv", [D, FH], DT, kind="ExternalInput")
    wo = nc.dram_tensor("wo", [FH, D], DT, kind="ExternalInput")
    cc = nc.dram_tensor("cc", [P, S], DT, kind="ExternalInput")
    ss = nc.dram_tensor("ss", [P, S], DT, kind="ExternalInput")
    masks = nc.dram_tensor("masks", [P, P], DT, kind="ExternalInput")
    resT = nc.dram_tensor("resT", [D, S], DT, kind="ExternalOutput")

    f32 = mybir.dt.float32
    Exp = mybir.ActivationFunctionType.Exp
    Copy = mybir.ActivationFunctionType.Copy

    with tile.TileContext(nc) as tc:
        with (
            tc.tile_pool(name="persist", bufs=1) as pp,
            tc.tile_pool(name="psA", bufs=4, space="PSUM") as psA,
            tc.tile_pool(name="psO", bufs=2, space="PSUM") as psO,
            tc.tile_pool(name="psD", bufs=2, space="PSUM") as psD,
        ):
            qT = pp.tile([P, HPC, S], DT)     # per head: rows=feat, free=seq
            kT = pp.tile([P, HPC, S], DT)
            vn = pp.tile([P, ST, FH], DT)     # v natural: [seq-block, feat]
            attnT = pp.tile([P, HPC, S], DT)  # normalized attention output^T
            cc_t = pp.tile([P, S], DT)
            ss_t = pp.tile([P, S], DT)
            mask_t = pp.tile([P, P], DT)      # tri mask: 1 if k <= q (local)
            ones_mat = pp.tile([P, P], DT)    # denominator reduce stationary
            wo_t = pp.tile([P, HPC, D], DT)

            nc.vector.memset(ones_mat, 1.0)

            # ---------------- Phase 1: projections + RoPE -----------------
            with (
                tc.tile_pool(name="wpool", bufs=1) as wp,
                tc.tile_pool(name="xpool", bufs=2) as xp,
                tc.tile_pool(name="ropetmp", bufs=6) as rp,
                tc.tile_pool(name="warm", bufs=1) as wmp,
            ):
                # HAM pre-warm: tiny matmuls on scratch data keep the PE
                # "busy" through the ~3.4us activity window while the input
                # DMAs stage, so the first real matmuls run at full clock.
                wsb = wmp.tile([P, 8], DT)
                wps = psA.tile([8, 8], f32, tag="ps", name="warm")
                nc.vector.memset(wsb, 0.0)
                for _ in range(96):
                    nc.tensor.matmul(wps, wsb, wsb, start=True, stop=True)

                wq_t = wp.tile([P, KT, FH], DT)
                wk_t = wp.tile([P, KT, FH], DT)
                wv_t = wp.tile([P, KT, FH], DT)
                # DMA issue order is the Sync-queue order: stage the first x
                # chunk and wq head-by-head so the first q-chain (head 0) can
                # start after ~2.5MB instead of 4MB; defer everything not
                # needed immediately.
                xc0 = xp.tile([P, KT, 512], DT, tag="xc", name="xc0")
                wqr = wq.rearrange("(kt p) f -> p kt f", p=P)
                for gs in (slice(0, 2), slice(2, 4), slice(4, 8),
                           slice(8, 12), slice(12, 16)):
                    nc.sync.dma_start(out=wq_t[:, gs, 0:DH], in_=wqr[:, gs, 0:DH])
                    nc.sync.dma_start(
                        out=xc0[:, gs, :],
                        in_=xT.rearrange("(kt p) s -> p kt s", p=P)[:, gs, 0:512])
                for h in range(1, HPC):
                    fsl = slice(h * DH, (h + 1) * DH)
                    nc.sync.dma_start(out=wq_t[:, :, fsl], in_=wqr[:, :, fsl])
                nc.sync.dma_start(out=cc_t[:, 0:512], in_=cc[:, 0:512])
                nc.sync.dma_start(out=ss_t[:, 0:512], in_=ss[:, 0:512])
                for g in range(4):
                    gs = slice(g * 4, (g + 1) * 4)
                    nc.sync.dma_start(
                        out=wk_t[:, gs, :],
                        in_=wk.rearrange("(kt p) f -> p kt f", p=P)[:, gs, :])
                nc.sync.dma_start(out=wv_t, in_=wv.rearrange("(kt p) f -> p kt f", p=P))
                nc.sync.dma_start(out=mask_t, in_=masks[:, :])
                nc.sync.dma_start(out=cc_t[:, 512:], in_=cc[:, 512:])
                nc.sync.dma_start(out=ss_t[:, 512:], in_=ss[:, 512:])

                for sc in range(SC):
                    if sc == 0:
                        xc = xc0
                    else:
                        xc = xp.tile([P, KT, 512], DT, tag="xc", name=f"xc{sc}")
                        for g in range(4):
                            gs = slice(g * 4, (g + 1) * 4)
                            nc.sync.dma_start(
                                out=xc[:, gs, :],
                                in_=xT.rearrange("(kt p) s -> p kt s", p=P)[
                                    :, gs, sc * 512:(sc + 1) * 512],
                            )
                    if sc == 1:
                        nc.sync.dma_start(
                            out=wo_t, in_=wo.rearrange("(ft p) d -> p ft d", p=P))
                    csl = slice(sc * 512, (sc + 1) * 512)
                    # q/k projections. RoPE fused into the PSUM drain:
                    # Scalar engine copies PSUM->fp16 SBUF, then three
                    # all-fp16 DVE ops (eligible for the fast DVE mode).
                    # ss_t rows 0:64 = +sin, rows 64:128 = -sin; the swap is
                    # done by writing each product into the opposite half.
                    for wt, dst in ((wq_t, qT), (wk_t, kT)):
                        for h in range(HPC):
                            fsl = slice(h * DH, (h + 1) * DH)
                            pq = psA.tile([P, 512], f32, tag="ps", name=f"pq{sc}{h}")
                            for k in range(KT):
                                nc.tensor.matmul(
                                    pq, wt[:, k, fsl], xc[:, k, :],
                                    start=(k == 0), stop=(k == KT - 1),
                                )
                            pqs = rp.tile([P, 512], DT, tag="pqs")
                            ta = rp.tile([P, 512], DT, tag="ta")
                            tb = rp.tile([P, 512], DT, tag="tb")
                            nc.scalar.activation(pqs, pq, Copy)
                            nc.vector.tensor_mul(ta, pqs, cc_t[:, csl])
                            nc.vector.tensor_mul(
                                tb[0:64, :], pqs[64:128, :], ss_t[64:128, csl])
                            nc.vector.tensor_mul(
                                tb[64:128, :], pqs[0:64, :], ss_t[0:64, csl])
                            nc.vector.tensor_add(dst[:, h, csl], ta, tb)
                    # v projection straight into natural layout
                    for st4 in range(4):
                        sb = sc * 4 + st4
                        pv = psA.tile([P, FH], f32, tag="ps", name=f"pv{sc}{st4}")
                        for k in range(KT):
                            nc.tensor.matmul(
                                pv, xc[:, k, st4 * P:(st4 + 1) * P], wv_t[:, k, :],
                                start=(k == 0), stop=(k == KT - 1),
                            )
                        nc.vector.tensor_copy(vn[:, sb, :], pv)

            # ---------- Phase 2+3: attention + interleaved out-proj -------
            # The out-projection of chunk qc is emitted after the attention
            # of chunk qc+1: by then its attnT inputs are long since ready,
            # so the in-order PE stream never stalls on the exp->normalize
            # dependency chain at a chunk boundary.
            with (
                tc.tile_pool(name="ppool", bufs=8) as ptp,
                tc.tile_pool(name="npool", bufs=4) as np_,
                tc.tile_pool(name="dpool", bufs=3) as dp,
                tc.tile_pool(name="rpool", bufs=2) as rop,
            ):
                resTr = resT.rearrange("(db p) s -> p db s", p=P)

                def attention_chunk(qc):
                    qsl = slice(qc * 512, (qc + 1) * 512)
                    for h in range(HPC):
                        fsl = slice(h * DH, (h + 1) * DH)
                        po = psO.tile([P, 512], f32, tag="po", name=f"po{h}{qc}")
                        pda = dp.tile([P, 512], DT, tag="pda", name=f"pda{h}{qc}")
                        nkb = 4 * qc + 4
                        for kb in range(nkb):
                            jl = kb - 4 * qc       # >=0 only on the diagonal
                            off = 128 * jl if jl > 0 else 0
                            wsl = slice(off, 512)
                            ps = psA.tile([P, 512], f32, tag="ps",
                                          name=f"ps{h}{qc}{kb}")
                            nc.tensor.matmul(
                                ps[:, wsl], kT[:, h, kb * P:(kb + 1) * P],
                                qT[:, h, qc * 512 + off:(qc + 1) * 512],
                                start=True, stop=True,
                            )
                            pt = ptp.tile([P, 512], DT, tag="pt")
                            nc.scalar.activation(
                                pt[:, wsl], ps[:, wsl], Exp, scale=float(SCALE))
                            if jl >= 0:
                                # triangular mask on the 128-wide diag window
                                nc.vector.tensor_mul(
                                    pt[:, off:off + P], pt[:, off:off + P],
                                    mask_t)
                            nc.tensor.matmul(
                                po[:, wsl], vn[:, kb, fsl], pt[:, wsl],
                                start=(kb == 0), stop=(kb == nkb - 1),
                                skip_group_check=True,
                            )
                            if kb == 0:
                                nc.vector.tensor_copy(pda, pt)
                            else:
                                nc.vector.tensor_add(
                                    pda[:, wsl], pda[:, wsl], pt[:, wsl])
                        pd = psD.tile([P, 512], f32, tag="pd", name=f"pd{h}{qc}")
                        nc.tensor.matmul(pd, ones_mat, pda, start=True, stop=True)
                        bc = np_.tile([P, 512], f32, tag="bc")
                        nc.vector.reciprocal_approx_fast(out=bc, in_=pd)
                        nc.vector.tensor_mul(attnT[:, h, qsl], po, bc)

                def outproj_chunk(qc):
                    qsl = slice(qc * 512, (qc + 1) * 512)
                    rt = rop.tile([P, KT, 512], DT, tag="rt", name=f"rt{qc}")
                    for db in range(KT):
                        pr = psA.tile([P, 512], f32, tag="ps", name=f"pr{qc}{db}")
                        for ft in range(HPC):
                            nc.tensor.matmul(
                                pr, wo_t[:, ft, db * P:(db + 1) * P],
                                attnT[:, ft, qsl],
                                start=(ft == 0), stop=(ft == HPC - 1),
                            )
                        if db % 4 == 3:
                            nc.scalar.activation(rt[:, db, :], pr, Copy)
                        else:
                            nc.vector.tensor_copy(rt[:, db, :], pr)
                        if db % 4 == 3:
                            nc.sync.dma_start(
                                out=resTr[:, db - 3:db + 1, qsl],
                                in_=rt[:, db - 3:db + 1, :])

                for qc in range(SC):
                    attention_chunk(qc)
                    if qc > 0:
                        outproj_chunk(qc - 1)
                outproj_chunk(SC - 1)

    nc.finalize()
    _prog_cache["nc"] = nc
    return nc


def _host_inputs(x, w_q, w_k, w_v, w_o):
    """Build the 8 per-core input maps."""
    # RoPE de-interleave permutation per head: evens then odds
    i = np.arange(DH)
    perm_head = np.concatenate([i[0::2], i[1::2]])  # within-head column order

    t = np.arange(S, dtype=np.float64)
    inv_freq = 1.0 / (THETA ** (np.arange(0, DH, 2, dtype=np.float64) / DH))
    ang = np.outer(t, inv_freq)          # [S, 64]
    cosT = np.cos(ang).T.astype(np.float32)   # [64, S]
    sinT = np.sin(ang).T.astype(np.float32)
    cc = np.vstack([cosT, cosT]).astype(NPDT)   # [128, S]
    ss = np.vstack([sinT, -sinT]).astype(NPDT)  # +sin feeds bottom half

    # shared diagonal mask: mask[k, q] = 1 if k <= q (128-wide local window)
    kk = np.arange(P)[:, None]
    qq = np.arange(P)[None, :]
    masks = (kk <= qq).astype(NPDT)      # [128, 128]

    in_maps = []
    for core in range(N_CORES):
        b = core // 4
        h0 = (core % 4) * HPC
        cols = np.concatenate(
            [h * DH + perm_head for h in range(h0, h0 + HPC)])   # rope-permuted
        vcols = np.arange(h0 * DH, (h0 + HPC) * DH)              # natural
        in_maps.append({
            "xT": np.ascontiguousarray(x[b].T).astype(NPDT),
            "wq": np.ascontiguousarray(w_q[:, cols]).astype(NPDT),
            "wk": np.ascontiguousarray(w_k[:, cols]).astype(NPDT),
            "wv": np.ascontiguousarray(w_v[:, vcols]).astype(NPDT),
            "wo": np.ascontiguousarray(w_o[vcols, :]).astype(NPDT),
            "cc": cc,
            "ss": ss,
            "masks": masks,
        })
    return in_maps


def kernel(x, w_q, w_k, w_v, w_o, _trace=False, _results_out=None):
    x = np.asarray(x, dtype=np.float32)
    w_q = np.asarray(w_q, dtype=np.float32)
    w_k = np.asarray(w_k, dtype=np.float32)
    w_v = np.asarray(w_v, dtype=np.float32)
    w_o = np.asarray(w_o, dtype=np.float32)
    nc = _build()
    in_maps = _host_inputs(x, w_q, w_k, w_v, w_o)
    res = run_bass_kernel_spmd(
        nc, in_maps, core_ids=list(range(N_CORES)), trace=_trace)
    if _results_out is not None:
        _results_out.append(res)
    out = np.empty((B, S, D), np.float32)
    for b in range(B):
        acc = res.results[4 * b]["resT"].astype(np.float32)
        for g in range(1, 4):
            acc = acc + res.results[4 * b + g]["resT"].astype(np.float32)
        out[b] = acc.T
    return out


# revision 11
# speedup vs baseline: 1.2819x; 1.0437x over previous
"""Llama MHA (B=2, S=2048, D=2048, H=16, causal, RoPE) on 8 trn2 cores.

Sharding: data-parallel over batch (2 groups of 4 cores) x tensor-parallel
over heads (4 heads per core). Each core computes, for its (batch, 4 heads):
  qT/kT = w^T x^T  (features on partitions, seq on free dim)
  RoPE on qT/kT (weights column-permuted on host so even/odd feature pairs
  land de-interleaved; dot products are permutation-invariant so scores
  match the reference exactly). RoPE runs as one Scalar-engine PSUM drain
  to fp16 followed by all-fp16 SBUF DVE ops (4x DVE mode).
  scoresT[k,q] blocks -> exp(fp16). Causal handling: diagonal-superblock
  matmuls are narrowed to the valid query range (saves ~10% of attention
  PE cycles) and a single shared [128,128] triangular mask covers the
  128-wide diagonal window of each key block.
  Softmax denominator: pt blocks are accumulated per (head, qchunk) with
  cheap fp16 DVE adds, then ONE [128,128]-ones matmul per (head, qchunk)
  does the partition reduction with a free PSUM row-broadcast (replaces
  the per-block ones-matmuls: ~82K -> ~8K PE cycles).
  Out-projection is interleaved per query chunk so the PE keeps working
  through the activation-bound attention stretches; partial results ship
  as fp16 and the host sums the 4 partials per batch in fp32.

All matmuls in fp16 (fp32 PSUM accumulation); softmax/normalization fp32.
"""

import numpy as np

import concourse.bass as bass
import concourse.mybir as mybir
import concourse.tile as tile
from concourse import bacc
from concourse.bass_utils import run_bass_kernel_spmd

B, S, D, H = 2, 2048, 2048, 16
DH = D // H            # 128 head dim
HPC = 4                # heads per core
N_CORES = 8
FH = HPC * DH          # 512 features per core
P = 128
KT = D // P            # 16 k-tiles over D
SC = S // 512          # 4 seq chunks of 512
ST = S // P            # 16 seq blocks of 128
THETA = 10000.0
SCALE = 1.0 / np.sqrt(DH)

DT = mybir.dt.float16
NPDT = np.float16

_prog_cache = {}


def _build():
    if "nc" in _prog_cache:
        return _prog_cache["nc"]
    nc = bacc.Bacc(None, target_bir_lowering=False, debug=False)

    xT = nc.dram_tensor("xT", [D, S], DT, kind="ExternalInput")
    wq = nc.dram_tensor("wq", [D, FH], DT, kind="ExternalInput")
    wk = nc.dram_tensor("wk", [D, FH], DT, kind="ExternalInput")
    wv = nc.dram_tensor("wv", [D, FH], DT, kind="ExternalInput")
    wo = nc.dram_tensor("wo", [FH, D], DT, kind="ExternalInput")
    cc = nc.dram_tensor("cc", [P, S], DT, kind="ExternalInput")
    ss = nc.dram_tensor("ss", [P, S], DT, kind="ExternalInput")
    masks = nc.dram_tensor("masks", [P, P], DT, kind="ExternalInput")
    resT = nc.dram_tensor("resT", [D, S], DT, kind="ExternalOutput")

    f32 = mybir.dt.float32
    Exp = mybir.ActivationFunctionType.Exp
    Copy = mybir.ActivationFunctionType.Copy

    with tile.TileContext(nc) as tc:
        with (
            tc.tile_pool(name="persist", bufs=1) as pp,
            tc.tile_pool(name="psA", bufs=4, space="PSUM") as psA,
            tc.tile_pool(name="psO", bufs=2, space="PSUM") as psO,
            tc.tile_pool(name="psD", bufs=2, space="PSUM") as psD,
        ):
            qT = pp.tile([P, HPC, S], DT)     # per head: rows=feat, free=seq
            kT = pp.tile([P, HPC, S], DT)
            vn = pp.tile([P, ST, FH], DT)     # v natural: [seq-block, feat]
            attnT = pp.tile([P, HPC, S], DT)  # normalized attention output^T
            cc_t = pp.tile([P, S], DT)
            ss_t = pp.tile([P, S], DT)
            mask_t = pp.tile([P, P], DT)      # tri mask: 1 if k <= q (local)
            ones_mat = pp.tile([P, P], DT)    # denominator reduce stationary
            wo_t = pp.tile([P, HPC, D], DT)

            nc.vector.memset(ones_mat, 1.0)

            # ---------------- Phase 1: projections + RoPE -----------------
            with (
                tc.tile_pool(name="wpool", bufs=1) as wp,
                tc.tile_pool(name="xpool", bufs=2) as xp,
                tc.tile_pool(name="ropetmp", bufs=6) as rp,
                tc.tile_pool(name="warm", bufs=1) as wmp,
            ):
                # HAM pre-warm: full-width matmuls on scratch data keep the
                # PE array genuinely busy through the ~3.4us activity window
                # while the input DMAs stage, so the first real matmuls run
                # at full clock. (Narrow matmuls don't register enough
                # activity — the array idles between dispatches.)
                wsb = wmp.tile([P, 512], DT)
                wps = psA.tile([P, 512], f32, tag="ps", name="warm")
                nc.vector.memset(wsb, 0.0)
                for _ in range(14):
                    nc.tensor.matmul(wps, wsb[:, 0:P], wsb, start=True, stop=True)

                wq_t = wp.tile([P, KT, FH], DT)
                wk_t = wp.tile([P, KT, FH], DT)
                wv_t = wp.tile([P, KT, FH], DT)
                # DMA issue order is the Sync-queue order: stage the first x
                # chunk and wq head-by-head so the first q-chain (head 0) can
                # start after ~2.5MB instead of 4MB; defer everything not
                # needed immediately.
                xc0 = xp.tile([P, KT, 512], DT, tag="xc", name="xc0")
                wqr = wq.rearrange("(kt p) f -> p kt f", p=P)
                for gs in (slice(0, 2), slice(2, 4), slice(4, 8),
                           slice(8, 12), slice(12, 16)):
                    nc.sync.dma_start(out=wq_t[:, gs, 0:DH], in_=wqr[:, gs, 0:DH])
                    nc.sync.dma_start(
                        out=xc0[:, gs, :],
                        in_=xT.rearrange("(kt p) s -> p kt s", p=P)[:, gs, 0:512])
                for h in range(1, HPC):
                    fsl = slice(h * DH, (h + 1) * DH)
                    nc.sync.dma_start(out=wq_t[:, :, fsl], in_=wqr[:, :, fsl])
                nc.sync.dma_start(out=cc_t[:, 0:512], in_=cc[:, 0:512])
                nc.sync.dma_start(out=ss_t[:, 0:512], in_=ss[:, 0:512])
                for g in range(4):
                    gs = slice(g * 4, (g + 1) * 4)
                    nc.sync.dma_start(
                        out=wk_t[:, gs, :],
                        in_=wk.rearrange("(kt p) f -> p kt f", p=P)[:, gs, :])
                nc.sync.dma_start(out=wv_t, in_=wv.rearrange("(kt p) f -> p kt f", p=P))
                nc.sync.dma_start(out=mask_t, in_=masks[:, :])
                nc.sync.dma_start(out=cc_t[:, 512:], in_=cc[:, 512:])
                nc.sync.dma_start(out=ss_t[:, 512:], in_=ss[:, 512:])

                for sc in range(SC):
                    if sc == 0:
                        xc = xc0
                    else:
                        xc = xp.tile([P, KT, 512], DT, tag="xc", name=f"xc{sc}")
                        for g in range(4):
                            gs = slice(g * 4, (g + 1) * 4)
                            nc.sync.dma_start(
                                out=xc[:, gs, :],
                                in_=xT.rearrange("(kt p) s -> p kt s", p=P)[
                                    :, gs, sc * 512:(sc + 1) * 512],
                            )
                    if sc == 1:
                        nc.sync.dma_start(
                            out=wo_t, in_=wo.rearrange("(ft p) d -> p ft d", p=P))
                    csl = slice(sc * 512, (sc + 1) * 512)
                    # q/k projections. RoPE fused into the PSUM drain:
                    # Scalar engine copies PSUM->fp16 SBUF, then three
                    # all-fp16 DVE ops (eligible for the fast DVE mode).
                    # ss_t rows 0:64 = +sin, rows 64:128 = -sin; the swap is
                    # done by writing each product into the opposite half.
                    for wt, dst in ((wq_t, qT), (wk_t, kT)):
                        for h in range(HPC):
                            fsl = slice(h * DH, (h + 1) * DH)
                            pq = psA.tile([P, 512], f32, tag="ps", name=f"pq{sc}{h}")
                            for k in range(KT):
                                nc.tensor.matmul(
                                    pq, wt[:, k, fsl], xc[:, k, :],
                                    start=(k == 0), stop=(k == KT - 1),
                                )
                            pqs = rp.tile([P, 512], DT, tag="pqs")
                            ta = rp.tile([P, 512], DT, tag="ta")
                            tb = rp.tile([P, 512], DT, tag="tb")
                            nc.scalar.activation(pqs, pq, Copy)
                            nc.vector.tensor_mul(ta, pqs, cc_t[:, csl])
                            nc.vector.tensor_mul(
                                tb[0:64, :], pqs[64:128, :], ss_t[64:128, csl])
                            nc.vector.tensor_mul(
                                tb[64:128, :], pqs[0:64, :], ss_t[0:64, csl])
                            nc.vector.tensor_add(dst[:, h, csl], ta, tb)
                    # v projection straight into natural layout
                    for st4 in range(4):
                        sb = sc * 4 + st4
                        pv = psA.tile([P, FH], f32, tag="ps", name=f"pv{sc}{st4}")
                        for k in range(KT):
                            nc.tensor.matmul(
                                pv, xc[:, k, st4 * P:(st4 + 1) * P], wv_t[:, k, :],
                                start=(k == 0), stop=(k == KT - 1),
                            )
                        nc.vector.tensor_copy(vn[:, sb, :], pv)

            # ---------- Phase 2+3: attention + interleaved out-proj -------
            # The attention inner loop is exp-throughput-bound on the Scalar
            # engine (~600ns/block vs ~430ns of PE work), so the PE would
            # stall on every PV matmul waiting for its exp. The previous
            # chunk's out-projection is dependency-free by then: its db
            # blocks are injected INTO the block loop (one per few blocks)
            # to fill those stalls. The final chunk's out-projection runs
            # standalone at the end (nothing left to overlap with).
            with (
                tc.tile_pool(name="ppool", bufs=8) as ptp,
                tc.tile_pool(name="npool", bufs=4) as np_,
                tc.tile_pool(name="dpool", bufs=3) as dp,
                tc.tile_pool(name="rpool", bufs=2) as rop,
            ):
                resTr = resT.rearrange("(db p) s -> p db s", p=P)

                def outproj_units(qc):
                    """One unit per db: 4 matmuls + drain (+ paired DMA)."""
                    qsl = slice(qc * 512, (qc + 1) * 512)
                    rt = rop.tile([P, KT, 512], DT, tag="rt", name=f"rt{qc}")

                    def unit(db):
                        pr = psA.tile([P, 512], f32, tag="ps", name=f"pr{qc}{db}")
                        for ft in range(HPC):
                            nc.tensor.matmul(
                                pr, wo_t[:, ft, db * P:(db + 1) * P],
                                attnT[:, ft, qsl],
                                start=(ft == 0), stop=(ft == HPC - 1),
                            )
                        if db % 4 == 3:
                            nc.scalar.activation(rt[:, db, :], pr, Copy)
                        else:
                            nc.vector.tensor_copy(rt[:, db, :], pr)
                        if db % 2 == 1:
                            nc.sync.dma_start(
                                out=resTr[:, db - 1:db + 1, qsl],
                                in_=rt[:, db - 1:db + 1, :])

                    return [lambda db=db: unit(db) for db in range(KT)]

                def attention_chunk(qc, inject):
                    qsl = slice(qc * 512, (qc + 1) * 512)
                    nkb = 4 * qc + 4
                    nblocks = HPC * nkb
                    # spread the injected units evenly across the block loop
                    inj_at = {}
                    for i, u in enumerate(inject):
                        inj_at.setdefault(
                            min(nblocks - 1, (i * nblocks) // len(inject) + 1),
                            []).append(u)
                    blk = 0
                    for h in range(HPC):
                        fsl = slice(h * DH, (h + 1) * DH)
                        po = psO.tile([P, 512], f32, tag="po", name=f"po{h}{qc}")
                        pda = dp.tile([P, 512], DT, tag="pda", name=f"pda{h}{qc}")
                        for kb in range(nkb):
                            jl = kb - 4 * qc       # >=0 only on the diagonal
                            off = 128 * jl if jl > 0 else 0
                            wsl = slice(off, 512)
                            ps = psA.tile([P, 512], f32, tag="ps",
                                          name=f"ps{h}{qc}{kb}")
                            nc.tensor.matmul(
                                ps[:, wsl], kT[:, h, kb * P:(kb + 1) * P],
                                qT[:, h, qc * 512 + off:(qc + 1) * 512],
                                start=True, stop=True,
                            )
                            pt = ptp.tile([P, 512], DT, tag="pt")
                            nc.scalar.activation(
                                pt[:, wsl], ps[:, wsl], Exp, scale=float(SCALE))
                            if jl >= 0:
                                # triangular mask on the 128-wide diag window
                                nc.vector.tensor_mul(
                                    pt[:, off:off + P], pt[:, off:off + P],
                                    mask_t)
                            nc.tensor.matmul(
                                po[:, wsl], vn[:, kb, fsl], pt[:, wsl],
                                start=(kb == 0), stop=(kb == nkb - 1),
                                skip_group_check=True,
                            )
                            if kb == 0:
                                nc.vector.tensor_copy(pda, pt)
                            else:
                                nc.vector.tensor_add(
                                    pda[:, wsl], pda[:, wsl], pt[:, wsl])
                            for u in inj_at.get(blk, ()):
                                u()
                            blk += 1
                        pd = psD.tile([P, 512], f32, tag="pd", name=f"pd{h}{qc}")
                        nc.tensor.matmul(pd, ones_mat, pda, start=True, stop=True)
                        bc = np_.tile([P, 512], f32, tag="bc")
                        nc.vector.reciprocal_approx_fast(out=bc, in_=pd)
                        nc.vector.tensor_mul(attnT[:, h, qsl], po, bc)

                for qc in range(SC):
                    attention_chunk(
                        qc, outproj_units(qc - 1) if qc > 0 else [])
                for u in outproj_units(SC - 1):
                    u()

    nc.finalize()
    _prog_cache["nc"] = nc
    return nc


def _host_inputs(x, w_q, w_k, w_v, w_o):
    """Build the 8 per-core input maps."""
    # RoPE de-interleave permutation per head: evens then odds
    i = np.arange(DH)
    perm_head = np.concatenate([i[0::2], i[1::2]])  # within-head column order

    t = np.arange(S, dtype=np.float64)
    inv_freq = 1.0 / (THETA ** (np.arange(0, DH, 2, dtype=np.float64) / DH))
    ang = np.outer(t, inv_freq)          # [S, 64]
    cosT = np.cos(ang).T.astype(np.float32)   # [64, S]
    sinT = np.sin(ang).T.astype(np.float32)
    cc = np.vstack([cosT, cosT]).astype(NPDT)   # [128, S]
    ss = np.vstack([sinT, -sinT]).astype(NPDT)  # +sin feeds bottom half

    # shared diagonal mask: mask[k, q] = 1 if k <= q (128-wide local window)
    kk = np.arange(P)[:, None]
    qq = np.arange(P)[None, :]
    masks = (kk <= qq).astype(NPDT)      # [128, 128]

    in_maps = []
    for core in range(N_CORES):
        b = core // 4
        h0 = (core % 4) * HPC
        cols = np.concatenate(
            [h * DH + perm_head for h in range(h0, h0 + HPC)])   # rope-permuted
        vcols = np.arange(h0 * DH, (h0 + HPC) * DH)              # natural
        in_maps.append({
            "xT": np.ascontiguousarray(x[b].T).astype(NPDT),
            "wq": np.ascontiguousarray(w_q[:, cols]).astype(NPDT),
            "wk": np.ascontiguousarray(w_k[:, cols]).astype(NPDT),
            "wv": np.ascontiguousarray(w_v[:, vcols]).astype(NPDT),
            "wo": np.ascontiguousarray(w_o[vcols, :]).astype(NPDT),
            "cc": cc,
            "ss": ss,
            "masks": masks,
        })
    return in_maps


def kernel(x, w_q, w_k, w_v, w_o, _trace=False, _results_out=None):
    x = np.asarray(x, dtype=np.float32)
    w_q = np.asarray(w_q, dtype=np.float32)
    w_k = np.asarray(w_k, dtype=np.float32)
    w_v = np.asarray(w_v, dtype=np.float32)
    w_o = np.asarray(w_o, dtype=np.float32)
    nc = _build()
    in_maps = _host_inputs(x, w_q, w_k, w_v, w_o)
    res = run_bass_kernel_spmd(
        nc, in_maps, core_ids=list(range(N_CORES)), trace=_trace)
    if _results_out is not None:
        _results_out.append(res)
    out = np.empty((B, S, D), np.float32)
    for b in range(B):
        acc = res.results[4 * b]["resT"].astype(np.float32)
        for g in range(1, 4):
            acc = acc + res.results[4 * b + g]["resT"].astype(np.float32)
        out[b] = acc.T
    return out


# revision 14
# speedup vs baseline: 1.3146x; 1.0255x over previous
"""Llama MHA (B=2, S=2048, D=2048, H=16, causal, RoPE) on 8 trn2 cores.

Sharding: data-parallel over batch (2 groups of 4 cores) x tensor-parallel
over heads (4 heads per core). Each core computes, for its (batch, 4 heads):
  qT/kT = w^T x^T  (features on partitions, seq on free dim)
  RoPE on qT/kT (weights column-permuted on host so even/odd feature pairs
  land de-interleaved; dot products are permutation-invariant so scores
  match the reference exactly). RoPE runs as one Scalar-engine PSUM drain
  to fp16 followed by all-fp16 SBUF DVE ops (4x DVE mode).
  scoresT[k,q] blocks -> exp(fp16). Causal handling: diagonal-superblock
  matmuls are narrowed to the valid query range (saves ~10% of attention
  PE cycles) and a single shared [128,128] triangular mask covers the
  128-wide diagonal window of each key block.
  Softmax denominator: pt blocks are accumulated per (head, qchunk) with
  cheap fp16 DVE adds, then ONE [128,128]-ones matmul per (head, qchunk)
  does the partition reduction with a free PSUM row-broadcast (replaces
  the per-block ones-matmuls: ~82K -> ~8K PE cycles).
  Out-projection is interleaved per query chunk so the PE keeps working
  through the activation-bound attention stretches; partial results ship
  as fp16 and the host sums the 4 partials per batch in fp32.

All matmuls in fp16 (fp32 PSUM accumulation); softmax/normalization fp32.
"""

import numpy as np

import concourse.bass as bass
import concourse.mybir as mybir
import concourse.tile as tile
from concourse import bacc
from concourse.bass_utils import run_bass_kernel_spmd

B, S, D, H = 2, 2048, 2048, 16
DH = D // H            # 128 head dim
HPC = 4                # heads per core
N_CORES = 8
FH = HPC * DH          # 512 features per core
P = 128
KT = D // P            # 16 k-tiles over D
SC = S // 512          # 4 seq chunks of 512
ST = S // P            # 16 seq blocks of 128
THETA = 10000.0
SCALE = 1.0 / np.sqrt(DH)

DT = mybir.dt.float16
NPDT = np.float16

_prog_cache = {}


def _build():
    if "nc" in _prog_cache:
        return _prog_cache["nc"]
    nc = bacc.Bacc(None, target_bir_lowering=False, debug=False)

    xT = nc.dram_tensor("xT", [D, S], DT, kind="ExternalInput")
    wq = nc.dram_tensor("wq", [D, FH], DT, kind="ExternalInput")
    wk = nc.dram_tensor("wk", [D, FH], DT, kind="ExternalInput")
    wv = nc.dram_tensor("wv", [D, FH], DT, kind="ExternalInput")
    wo = nc.dram_tensor("wo", [FH, D], DT, kind="ExternalInput")
    cc = nc.dram_tensor("cc", [P, S], DT, kind="ExternalInput")
    ss = nc.dram_tensor("ss", [P, S], DT, kind="ExternalInput")
    masks = nc.dram_tensor("masks", [P, P], DT, kind="ExternalInput")
    resT = nc.dram_tensor("resT", [D, S], DT, kind="ExternalOutput")

    f32 = mybir.dt.float32
    Exp = mybir.ActivationFunctionType.Exp
    Copy = mybir.ActivationFunctionType.Copy

    with tile.TileContext(nc) as tc:
        with (
            tc.tile_pool(name="persist", bufs=1) as pp,
            tc.tile_pool(name="psA", bufs=5, space="PSUM") as psA,
            tc.tile_pool(name="psO", bufs=2, space="PSUM") as psO,
            tc.tile_pool(name="psD", bufs=1, space="PSUM") as psD,
        ):
            qT = pp.tile([P, HPC, S], DT)     # per head: rows=feat, free=seq
            kT = pp.tile([P, HPC, S], DT)
            vn = pp.tile([P, ST, FH], DT)     # v natural: [seq-block, feat]
            attnT = pp.tile([P, HPC, S], DT)  # normalized attention output^T
            cc_t = pp.tile([P, S], DT)
            ss_t = pp.tile([P, S], DT)
            mask_t = pp.tile([P, P], DT)      # tri mask: 1 if k <= q (local)
            ones_mat = pp.tile([P, P], DT)    # denominator reduce stationary
            wo_t = pp.tile([P, HPC, D], DT)

            nc.vector.memset(ones_mat, 1.0)

            # ---------------- Phase 1: projections + RoPE -----------------
            with (
                tc.tile_pool(name="wpool", bufs=1) as wp,
                tc.tile_pool(name="xpool", bufs=2) as xp,
                tc.tile_pool(name="ropetmp", bufs=6) as rp,
                tc.tile_pool(name="warm", bufs=1) as wmp,
            ):
                # HAM pre-warm: full-width matmuls on scratch data keep the
                # PE array genuinely busy through the ~3.4us activity window
                # while the input DMAs stage, so the first real matmuls run
                # at full clock. (Narrow matmuls don't register enough
                # activity — the array idles between dispatches.)
                wsb = wmp.tile([P, 256], DT)
                wps = psA.tile([P, 256], f32, tag="ps", name="warm")
                nc.vector.memset(wsb, 0.0)
                for _ in range(16):
                    nc.tensor.matmul(wps, wsb[:, 0:P], wsb, start=True, stop=True)

                wq_t = wp.tile([P, KT, FH], DT)
                wk_t = wp.tile([P, KT, FH], DT)
                wv_t = wp.tile([P, KT, FH], DT)
                # DMA issue order is the Sync-queue order: stage the first x
                # chunk and wq head-by-head so the first q-chain (head 0) can
                # start after ~2.5MB instead of 4MB; defer everything not
                # needed immediately.
                xc0 = xp.tile([P, KT, 512], DT, tag="xc", name="xc0")
                wqr = wq.rearrange("(kt p) f -> p kt f", p=P)
                for gs in (slice(0, 2), slice(2, 4), slice(4, 8),
                           slice(8, 12), slice(12, 16)):
                    nc.sync.dma_start(out=wq_t[:, gs, 0:DH], in_=wqr[:, gs, 0:DH])
                    nc.sync.dma_start(
                        out=xc0[:, gs, :],
                        in_=xT.rearrange("(kt p) s -> p kt s", p=P)[:, gs, 0:512])
                for h in range(1, HPC):
                    fsl = slice(h * DH, (h + 1) * DH)
                    nc.sync.dma_start(out=wq_t[:, :, fsl], in_=wqr[:, :, fsl])
                nc.sync.dma_start(out=cc_t[:, 0:512], in_=cc[:, 0:512])
                nc.sync.dma_start(out=ss_t[:, 0:512], in_=ss[:, 0:512])
                for g in range(4):
                    gs = slice(g * 4, (g + 1) * 4)
                    nc.sync.dma_start(
                        out=wk_t[:, gs, :],
                        in_=wk.rearrange("(kt p) f -> p kt f", p=P)[:, gs, :])
                nc.sync.dma_start(out=wv_t, in_=wv.rearrange("(kt p) f -> p kt f", p=P))
                nc.sync.dma_start(out=mask_t, in_=masks[:, :])
                nc.sync.dma_start(out=cc_t[:, 512:], in_=cc[:, 512:])
                nc.sync.dma_start(out=ss_t[:, 512:], in_=ss[:, 512:])

                for sc in range(SC):
                    if sc == 0:
                        xc = xc0
                    else:
                        xc = xp.tile([P, KT, 512], DT, tag="xc", name=f"xc{sc}")
                        for g in range(4):
                            gs = slice(g * 4, (g + 1) * 4)
                            nc.sync.dma_start(
                                out=xc[:, gs, :],
                                in_=xT.rearrange("(kt p) s -> p kt s", p=P)[
                                    :, gs, sc * 512:(sc + 1) * 512],
                            )
                    if sc == 1:
                        nc.sync.dma_start(
                            out=wo_t, in_=wo.rearrange("(ft p) d -> p ft d", p=P))
                    csl = slice(sc * 512, (sc + 1) * 512)
                    # q/k projections. RoPE fused into the PSUM drain:
                    # Scalar engine copies PSUM->fp16 SBUF, then three
                    # all-fp16 DVE ops (eligible for the fast DVE mode).
                    # ss_t rows 0:64 = +sin, rows 64:128 = -sin; the swap is
                    # done by writing each product into the opposite half.
                    for wt, dst in ((wq_t, qT), (wk_t, kT)):
                        for h in range(HPC):
                            fsl = slice(h * DH, (h + 1) * DH)
                            pq = psA.tile([P, 512], f32, tag="ps", name=f"pq{sc}{h}")
                            for k in range(KT):
                                nc.tensor.matmul(
                                    pq, wt[:, k, fsl], xc[:, k, :],
                                    start=(k == 0), stop=(k == KT - 1),
                                )
                            pqs = rp.tile([P, 512], DT, tag="pqs")
                            ta = rp.tile([P, 512], DT, tag="ta")
                            tb = rp.tile([P, 512], DT, tag="tb")
                            nc.scalar.activation(pqs, pq, Copy)
                            nc.vector.tensor_mul(ta, pqs, cc_t[:, csl])
                            nc.vector.tensor_mul(
                                tb[0:64, :], pqs[64:128, :], ss_t[64:128, csl])
                            nc.vector.tensor_mul(
                                tb[64:128, :], pqs[0:64, :], ss_t[0:64, csl])
                            nc.vector.tensor_add(dst[:, h, csl], ta, tb)
                    # v projection straight into natural layout
                    for st4 in range(4):
                        sb = sc * 4 + st4
                        pv = psA.tile([P, FH], f32, tag="ps", name=f"pv{sc}{st4}")
                        for k in range(KT):
                            nc.tensor.matmul(
                                pv, xc[:, k, st4 * P:(st4 + 1) * P], wv_t[:, k, :],
                                start=(k == 0), stop=(k == KT - 1),
                            )
                        nc.vector.tensor_copy(vn[:, sb, :], pv)

            # ---------- Phase 2+3: attention + interleaved out-proj -------
            # The attention inner loop is exp-throughput-bound on the Scalar
            # engine (~600ns/block vs ~430ns of PE work), so the PE would
            # stall on every PV matmul waiting for its exp. The previous
            # chunk's out-projection is dependency-free by then: its db
            # blocks are injected INTO the block loop (one per few blocks)
            # to fill those stalls. The final chunk's out-projection runs
            # standalone at the end (nothing left to overlap with).
            with (
                tc.tile_pool(name="ppool", bufs=8) as ptp,
                tc.tile_pool(name="npool", bufs=4) as np_,
                tc.tile_pool(name="rpool", bufs=2) as rop,
            ):
                dp = np_
                resTr = resT.rearrange("(db p) s -> p db s", p=P)

                def outproj_units(qc):
                    """One unit per db: 4 matmuls + drain (+ paired DMA)."""
                    qsl = slice(qc * 512, (qc + 1) * 512)
                    rt = rop.tile([P, KT, 512], DT, tag="rt", name=f"rt{qc}")

                    def unit(db):
                        pr = psA.tile([P, 512], f32, tag="ps", name=f"pr{qc}{db}")
                        for ft in range(HPC):
                            nc.tensor.matmul(
                                pr, wo_t[:, ft, db * P:(db + 1) * P],
                                attnT[:, ft, qsl],
                                start=(ft == 0), stop=(ft == HPC - 1),
                            )
                        if db % 4 == 3:
                            nc.scalar.activation(rt[:, db, :], pr, Copy)
                        else:
                            nc.vector.tensor_copy(rt[:, db, :], pr)
                        if db % 2 == 1:
                            nc.sync.dma_start(
                                out=resTr[:, db - 1:db + 1, qsl],
                                in_=rt[:, db - 1:db + 1, :])

                    return [lambda db=db: unit(db) for db in range(KT)]

                def attention_chunk(qc, inject):
                    qsl = slice(qc * 512, (qc + 1) * 512)
                    nkb = 4 * qc + 4
                    nblocks = HPC * nkb
                    # spread the injected units evenly across the block loop
                    inj_at = {}
                    for i, u in enumerate(inject):
                        inj_at.setdefault(
                            min(nblocks - 1, (i * nblocks) // len(inject) + 1),
                            []).append(u)
                    blk = 0
                    for h in range(HPC):
                        fsl = slice(h * DH, (h + 1) * DH)
                        po = psO.tile([P, 512], f32, tag="po", name=f"po{h}{qc}")
                        pda = dp.tile([P, 512], DT, tag="pda", name=f"pda{h}{qc}")
                        for kb in range(nkb):
                            jl = kb - 4 * qc       # >=0 only on the diagonal
                            off = 128 * jl if jl > 0 else 0
                            wsl = slice(off, 512)
                            ps = psA.tile([P, 512], f32, tag="ps",
                                          name=f"ps{h}{qc}{kb}")
                            nc.tensor.matmul(
                                ps[:, wsl], kT[:, h, kb * P:(kb + 1) * P],
                                qT[:, h, qc * 512 + off:(qc + 1) * 512],
                                start=True, stop=True,
                            )
                            pt = ptp.tile([P, 512], DT, tag="pt")
                            nc.scalar.activation(
                                pt[:, wsl], ps[:, wsl], Exp, scale=float(SCALE))
                            if jl >= 0:
                                # triangular mask on the 128-wide diag window
                                nc.vector.tensor_mul(
                                    pt[:, off:off + P], pt[:, off:off + P],
                                    mask_t)
                            nc.tensor.matmul(
                                po[:, wsl], vn[:, kb, fsl], pt[:, wsl],
                                start=(kb == 0), stop=(kb == nkb - 1),
                                skip_group_check=True,
                            )
                            if kb == 0:
                                nc.vector.tensor_copy(pda, pt)
                            else:
                                nc.vector.tensor_add(
                                    pda[:, wsl], pda[:, wsl], pt[:, wsl])
                            for u in inj_at.get(blk, ()):
                                u()
                            blk += 1
                        pd = psD.tile([P, 512], f32, tag="pd", name=f"pd{h}{qc}")
                        nc.tensor.matmul(pd, ones_mat, pda, start=True, stop=True)
                        bc = np_.tile([P, 512], f32, tag="bc")
                        nc.vector.reciprocal_approx_fast(out=bc, in_=pd)
                        nc.vector.tensor_mul(attnT[:, h, qsl], po, bc)

                for qc in range(SC):
                    attention_chunk(
                        qc, outproj_units(qc - 1) if qc > 0 else [])
                for u in outproj_units(SC - 1):
                    u()

    nc.finalize()
    _prog_cache["nc"] = nc
    return nc


def _host_inputs(x, w_q, w_k, w_v, w_o):
    """Build the 8 per-core input maps."""
    # RoPE de-interleave permutation per head: evens then odds
    i = np.arange(DH)
    perm_head = np.concatenate([i[0::2], i[1::2]])  # within-head column order

    t = np.arange(S, dtype=np.float64)
    inv_freq = 1.0 / (THETA ** (np.arange(0, DH, 2, dtype=np.float64) / DH))
    ang = np.outer(t, inv_freq)          # [S, 64]
    cosT = np.cos(ang).T.astype(np.float32)   # [64, S]
    sinT = np.sin(ang).T.astype(np.float32)
    cc = np.vstack([cosT, cosT]).astype(NPDT)   # [128, S]
    ss = np.vstack([sinT, -sinT]).astype(NPDT)  # +sin feeds bottom half

    # shared diagonal mask: mask[k, q] = 1 if k <= q (128-wide local window)
    kk = np.arange(P)[:, None]
    qq = np.arange(P)[None, :]
    masks = (kk <= qq).astype(NPDT)      # [128, 128]

    in_maps = []
    for core in range(N_CORES):
        b = core // 4
        h0 = (core % 4) * HPC
        cols = np.concatenate(
            [h * DH + perm_head for h in range(h0, h0 + HPC)])   # rope-permuted
        vcols = np.arange(h0 * DH, (h0 + HPC) * DH)              # natural
        in_maps.append({
            "xT": np.ascontiguousarray(x[b].T).astype(NPDT),
            "wq": np.ascontiguousarray(w_q[:, cols]).astype(NPDT),
            "wk": np.ascontiguousarray(w_k[:, cols]).astype(NPDT),
            "wv": np.ascontiguousarray(w_v[:, vcols]).astype(NPDT),
            "wo": np.ascontiguousarray(w_o[vcols, :]).astype(NPDT),
            "cc": cc,
            "ss": ss,
            "masks": masks,
        })
    return in_maps


def kernel(x, w_q, w_k, w_v, w_o, _trace=False, _results_out=None):
    x = np.asarray(x, dtype=np.float32)
    w_q = np.asarray(w_q, dtype=np.float32)
    w_k = np.asarray(w_k, dtype=np.float32)
    w_v = np.asarray(w_v, dtype=np.float32)
    w_o = np.asarray(w_o, dtype=np.float32)
    nc = _build()
    in_maps = _host_inputs(x, w_q, w_k, w_v, w_o)
    res = run_bass_kernel_spmd(
        nc, in_maps, core_ids=list(range(N_CORES)), trace=_trace)
    if _results_out is not None:
        _results_out.append(res)
    out = np.empty((B, S, D), np.float32)
    for b in range(B):
        acc = res.results[4 * b]["resT"].astype(np.float32)
        for g in range(1, 4):
            acc = acc + res.results[4 * b + g]["resT"].astype(np.float32)
        out[b] = acc.T
    return out


# revision 16
# speedup vs baseline: 1.3146x; 1.0000x over previous
"""Llama MHA (B=2, S=2048, D=2048, H=16, causal, RoPE) on 8 trn2 cores.

Sharding: data-parallel over batch (2 groups of 4 cores) x tensor-parallel
over heads (4 heads per core). Each core computes, for its (batch, 4 heads):
  qT/kT = w^T x^T  (features on partitions, seq on free dim)
  RoPE on qT/kT (weights column-permuted on host so even/odd feature pairs
  land de-interleaved; dot products are permutation-invariant so scores
  match the reference exactly). RoPE runs as one Scalar-engine PSUM drain
  to fp16 followed by all-fp16 SBUF DVE ops (4x DVE mode).
  scoresT[k,q] blocks -> exp(fp16). Causal handling: diagonal-superblock
  matmuls are narrowed to the valid query range (saves ~10% of attention
  PE cycles) and a single shared [128,128] triangular mask covers the
  128-wide diagonal window of each key block.
  Softmax denominator: pt blocks are accumulated per (head, qchunk) with
  cheap fp16 DVE adds, then ONE [128,128]-ones matmul per (head, qchunk)
  does the partition reduction with a free PSUM row-broadcast (replaces
  the per-block ones-matmuls: ~82K -> ~8K PE cycles).
  Out-projection is interleaved per query chunk so the PE keeps working
  through the activation-bound attention stretches; partial results ship
  as fp16 and the host sums the 4 partials per batch in fp32.

All matmuls in fp16 (fp32 PSUM accumulation); softmax/normalization fp32.
"""

import numpy as np

import concourse.bass as bass
import concourse.mybir as mybir
import concourse.tile as tile
from concourse import bacc
from concourse.bass_utils import run_bass_kernel_spmd

B, S, D, H = 2, 2048, 2048, 16
DH = D // H            # 128 head dim
HPC = 4                # heads per core
N_CORES = 8
FH = HPC * DH          # 512 features per core
P = 128
KT = D // P            # 16 k-tiles over D
SC = S // 512          # 4 seq chunks of 512
ST = S // P            # 16 seq blocks of 128
THETA = 10000.0
SCALE = 1.0 / np.sqrt(DH)

DT = mybir.dt.float16
NPDT = np.float16

_prog_cache = {}


def _build():
    if "nc" in _prog_cache:
        return _prog_cache["nc"]
    nc = bacc.Bacc(None, target_bir_lowering=False, debug=False)

    xT = nc.dram_tensor("xT", [D, S], DT, kind="ExternalInput")
    wq = nc.dram_tensor("wq", [D, FH], DT, kind="ExternalInput")
    wk = nc.dram_tensor("wk", [D, FH], DT, kind="ExternalInput")
    wv = nc.dram_tensor("wv", [D, FH], DT, kind="ExternalInput")
    wo = nc.dram_tensor("wo", [FH, D], DT, kind="ExternalInput")
    cc = nc.dram_tensor("cc", [P, S], DT, kind="ExternalInput")
    ss = nc.dram_tensor("ss", [P, S], DT, kind="ExternalInput")
    masks = nc.dram_tensor("masks", [P, P], DT, kind="ExternalInput")
    resT = nc.dram_tensor("resT", [D, S], DT, kind="ExternalOutput")

    f32 = mybir.dt.float32
    Exp = mybir.ActivationFunctionType.Exp
    Copy = mybir.ActivationFunctionType.Copy

    with tile.TileContext(nc) as tc:
        with (
            tc.tile_pool(name="persist", bufs=1) as pp,
            tc.tile_pool(name="psA", bufs=5, space="PSUM") as psA,
            tc.tile_pool(name="psO", bufs=2, space="PSUM") as psO,
            tc.tile_pool(name="psD", bufs=1, space="PSUM") as psD,
        ):
            qT = pp.tile([P, HPC, S], DT)     # per head: rows=feat, free=seq
            kT = pp.tile([P, HPC, S], DT)
            vn = pp.tile([P, ST, FH], DT)     # v natural: [seq-block, feat]
            attnT = pp.tile([P, HPC, S], DT)  # normalized attention output^T
            cc_t = pp.tile([P, S], DT)
            ss_t = pp.tile([P, S], DT)
            mask_t = pp.tile([P, P], DT)      # tri mask: 1 if k <= q (local)
            ones_mat = pp.tile([P, P], DT)    # denominator reduce stationary
            wo_t = pp.tile([P, HPC, D], DT)

            nc.vector.memset(ones_mat, 1.0)

            # ---------------- Phase 1: projections + RoPE -----------------
            with (
                tc.tile_pool(name="wpool", bufs=1) as wp,
                tc.tile_pool(name="xpool", bufs=2) as xp,
                tc.tile_pool(name="ropetmp", bufs=6) as rp,
                tc.tile_pool(name="warm", bufs=1) as wmp,
            ):
                # HAM pre-warm: full-width matmuls on scratch data keep the
                # PE array genuinely busy through the ~3.4us activity window
                # while the input DMAs stage, so the first real matmuls run
                # at full clock. (Narrow matmuls don't register enough
                # activity — the array idles between dispatches.)
                wsb = wmp.tile([P, 256], DT)
                wps = psA.tile([P, 256], f32, tag="ps", name="warm")
                nc.vector.memset(wsb, 0.0)
                for _ in range(16):
                    nc.tensor.matmul(wps, wsb[:, 0:P], wsb, start=True, stop=True)

                wq_t = wp.tile([P, KT, FH], DT)
                wk_t = wp.tile([P, KT, FH], DT)
                wv_t = wp.tile([P, KT, FH], DT)
                # DMA issue order is the Sync-queue order: stage the first x
                # chunk and wq head-by-head so the first q-chain (head 0) can
                # start after ~2.5MB instead of 4MB; defer everything not
                # needed immediately.
                xc0 = xp.tile([P, KT, 512], DT, tag="xc", name="xc0")
                wqr = wq.rearrange("(kt p) f -> p kt f", p=P)
                for gs in (slice(0, 2), slice(2, 4), slice(4, 8),
                           slice(8, 12), slice(12, 16)):
                    nc.sync.dma_start(out=wq_t[:, gs, 0:DH], in_=wqr[:, gs, 0:DH])
                    nc.sync.dma_start(
                        out=xc0[:, gs, :],
                        in_=xT.rearrange("(kt p) s -> p kt s", p=P)[:, gs, 0:512])
                for h in range(1, HPC):
                    fsl = slice(h * DH, (h + 1) * DH)
                    nc.sync.dma_start(out=wq_t[:, :, fsl], in_=wqr[:, :, fsl])
                nc.sync.dma_start(out=cc_t[:, 0:512], in_=cc[:, 0:512])
                nc.sync.dma_start(out=ss_t[:, 0:512], in_=ss[:, 0:512])
                for g in range(4):
                    gs = slice(g * 4, (g + 1) * 4)
                    nc.sync.dma_start(
                        out=wk_t[:, gs, :],
                        in_=wk.rearrange("(kt p) f -> p kt f", p=P)[:, gs, :])
                nc.sync.dma_start(out=wv_t, in_=wv.rearrange("(kt p) f -> p kt f", p=P))
                nc.sync.dma_start(out=mask_t, in_=masks[:, :])
                nc.sync.dma_start(out=cc_t[:, 512:], in_=cc[:, 512:])
                nc.sync.dma_start(out=ss_t[:, 512:], in_=ss[:, 512:])

                for sc in range(SC):
                    if sc == 0:
                        xc = xc0
                    else:
                        xc = xp.tile([P, KT, 512], DT, tag="xc", name=f"xc{sc}")
                        for g in range(4):
                            gs = slice(g * 4, (g + 1) * 4)
                            nc.sync.dma_start(
                                out=xc[:, gs, :],
                                in_=xT.rearrange("(kt p) s -> p kt s", p=P)[
                                    :, gs, sc * 512:(sc + 1) * 512],
                            )
                    if sc == 1:
                        nc.sync.dma_start(
                            out=wo_t, in_=wo.rearrange("(ft p) d -> p ft d", p=P))
                    csl = slice(sc * 512, (sc + 1) * 512)
                    # q/k projections. RoPE fused into the PSUM drain:
                    # Scalar engine copies PSUM->fp16 SBUF, then three
                    # all-fp16 DVE ops (eligible for the fast DVE mode).
                    # ss_t rows 0:64 = +sin, rows 64:128 = -sin; the swap is
                    # done by writing each product into the opposite half.
                    for wt, dst in ((wq_t, qT), (wk_t, kT)):
                        for h in range(HPC):
                            fsl = slice(h * DH, (h + 1) * DH)
                            pq = psA.tile([P, 512], f32, tag="ps", name=f"pq{sc}{h}")
                            for k in range(KT):
                                nc.tensor.matmul(
                                    pq, wt[:, k, fsl], xc[:, k, :],
                                    start=(k == 0), stop=(k == KT - 1),
                                )
                            pqs = rp.tile([P, 512], DT, tag="pqs")
                            ta = rp.tile([P, 512], DT, tag="ta")
                            tb = rp.tile([P, 512], DT, tag="tb")
                            nc.scalar.activation(pqs, pq, Copy)
                            nc.vector.tensor_mul(ta, pqs, cc_t[:, csl])
                            nc.vector.tensor_mul(
                                tb[0:64, :], pqs[64:128, :], ss_t[64:128, csl])
                            nc.vector.tensor_mul(
                                tb[64:128, :], pqs[0:64, :], ss_t[0:64, csl])
                            nc.vector.tensor_add(dst[:, h, csl], ta, tb)
                    # v projection straight into natural layout
                    for st4 in range(4):
                        sb = sc * 4 + st4
                        pv = psA.tile([P, FH], f32, tag="ps", name=f"pv{sc}{st4}")
                        for k in range(KT):
                            nc.tensor.matmul(
                                pv, xc[:, k, st4 * P:(st4 + 1) * P], wv_t[:, k, :],
                                start=(k == 0), stop=(k == KT - 1),
                            )
                        nc.vector.tensor_copy(vn[:, sb, :], pv)

            # ---------- Phase 2+3: attention + interleaved out-proj -------
            # The attention inner loop is exp-throughput-bound on the Scalar
            # engine (~600ns/block vs ~430ns of PE work), so the PE would
            # stall on every PV matmul waiting for its exp. The previous
            # chunk's out-projection is dependency-free by then: its db
            # blocks are injected INTO the block loop (one per few blocks)
            # to fill those stalls. The final chunk's out-projection runs
            # standalone at the end (nothing left to overlap with).
            with (
                tc.tile_pool(name="ppool", bufs=8) as ptp,
                tc.tile_pool(name="npool", bufs=4) as np_,
                tc.tile_pool(name="rpool", bufs=2) as rop,
            ):
                dp = np_
                resTr = resT.rearrange("(db p) s -> p db s", p=P)

                def outproj_units(qc):
                    """One unit per db: 4 matmuls + drain (+ paired DMA)."""
                    qsl = slice(qc * 512, (qc + 1) * 512)
                    rt = rop.tile([P, KT, 512], DT, tag="rt", name=f"rt{qc}")

                    def unit(db):
                        pr = psA.tile([P, 512], f32, tag="ps", name=f"pr{qc}{db}")
                        for ft in range(HPC):
                            nc.tensor.matmul(
                                pr, wo_t[:, ft, db * P:(db + 1) * P],
                                attnT[:, ft, qsl],
                                start=(ft == 0), stop=(ft == HPC - 1),
                            )
                        if db % 4 == 3:
                            nc.scalar.activation(rt[:, db, :], pr, Copy)
                        else:
                            nc.vector.tensor_copy(rt[:, db, :], pr)
                        if db % 2 == 1:
                            nc.sync.dma_start(
                                out=resTr[:, db - 1:db + 1, qsl],
                                in_=rt[:, db - 1:db + 1, :])

                    return [lambda db=db: unit(db) for db in range(KT)]

                # Depth-2 software pipeline: block k's PV matmul is emitted
                # during block k+1 (after its score matmul and any injected
                # out-proj unit), so the PE never sits in-order behind the
                # ~600ns exp. Each head's denominator finalize (ones-matmul,
                # reciprocal, normalize) is likewise deferred into the next
                # head. `pend` carries across heads and chunks.
                pend = {"pv": None, "fin": None}

                def flush_pv():
                    if pend["pv"] is not None:
                        pend["pv"]()
                        pend["pv"] = None

                def flush_fin():
                    if pend["fin"] is not None:
                        pend["fin"]()
                        pend["fin"] = None

                def attention_chunk(qc, inject):
                    qsl = slice(qc * 512, (qc + 1) * 512)
                    nkb = 4 * qc + 4
                    nblocks = HPC * nkb
                    # spread the injected units evenly across the block loop
                    inj_at = {}
                    for i, u in enumerate(inject):
                        inj_at.setdefault(
                            min(nblocks - 1, (i * nblocks) // len(inject) + 1),
                            []).append(u)
                    blk = 0
                    for h in range(HPC):
                        fsl = slice(h * DH, (h + 1) * DH)
                        po = psO.tile([P, 512], f32, tag="po", name=f"po{h}{qc}")
                        pda = dp.tile([P, 512], DT, tag="pda", name=f"pda{h}{qc}")
                        for kb in range(nkb):
                            jl = kb - 4 * qc       # >=0 only on the diagonal
                            off = 128 * jl if jl > 0 else 0
                            wsl = slice(off, 512)
                            ps = psA.tile([P, 512], f32, tag="ps",
                                          name=f"ps{h}{qc}{kb}")
                            nc.tensor.matmul(
                                ps[:, wsl], kT[:, h, kb * P:(kb + 1) * P],
                                qT[:, h, qc * 512 + off:(qc + 1) * 512],
                                start=True, stop=True,
                            )
                            pt = ptp.tile([P, 512], DT, tag="pt")
                            nc.scalar.activation(
                                pt[:, wsl], ps[:, wsl], Exp, scale=float(SCALE))
                            if jl >= 0:
                                # triangular mask on the 128-wide diag window
                                nc.vector.tensor_mul(
                                    pt[:, off:off + P], pt[:, off:off + P],
                                    mask_t)
                            flush_pv()
                            if kb == 1:
                                # previous head's/chunk's finalize runs (and
                                # writes attnT) before any injected out-proj
                                # unit can read that attnT chunk
                                flush_fin()
                            for u in inj_at.get(blk, ()):
                                u()

                            def pv_unit(po=po, pda=pda, pt=pt, kb=kb, wsl=wsl,
                                        fsl=fsl, nkb=nkb):
                                nc.tensor.matmul(
                                    po[:, wsl], vn[:, kb, fsl], pt[:, wsl],
                                    start=(kb == 0), stop=(kb == nkb - 1),
                                    skip_group_check=True,
                                )
                                if kb == 0:
                                    nc.vector.tensor_copy(pda, pt)
                                else:
                                    nc.vector.tensor_add(
                                        pda[:, wsl], pda[:, wsl], pt[:, wsl])

                            pend["pv"] = pv_unit
                            blk += 1

                        def fin_unit(h=h, qc=qc, po=po, pda=pda, qsl=qsl):
                            pd = psD.tile([P, 512], f32, tag="pd",
                                          name=f"pd{h}{qc}")
                            nc.tensor.matmul(pd, ones_mat, pda,
                                             start=True, stop=True)
                            bc = np_.tile([P, 512], f32, tag="bc")
                            nc.vector.reciprocal_approx_fast(out=bc, in_=pd)
                            nc.vector.tensor_mul(attnT[:, h, qsl], po, bc)

                        flush_fin()
                        pend["fin"] = fin_unit

                for qc in range(SC):
                    attention_chunk(
                        qc, outproj_units(qc - 1) if qc > 0 else [])
                flush_pv()
                flush_fin()
                for u in outproj_units(SC - 1):
                    u()

    nc.finalize()
    _prog_cache["nc"] = nc
    return nc


def _host_inputs(x, w_q, w_k, w_v, w_o):
    """Build the 8 per-core input maps."""
    # RoPE de-interleave permutation per head: evens then odds
    i = np.arange(DH)
    perm_head = np.concatenate([i[0::2], i[1::2]])  # within-head column order

    t = np.arange(S, dtype=np.float64)
    inv_freq = 1.0 / (THETA ** (np.arange(0, DH, 2, dtype=np.float64) / DH))
    ang = np.outer(t, inv_freq)          # [S, 64]
    cosT = np.cos(ang).T.astype(np.float32)   # [64, S]
    sinT = np.sin(ang).T.astype(np.float32)
    cc = np.vstack([cosT, cosT]).astype(NPDT)   # [128, S]
    ss = np.vstack([sinT, -sinT]).astype(NPDT)  # +sin feeds bottom half

    # shared diagonal mask: mask[k, q] = 1 if k <= q (128-wide local window)
    kk = np.arange(P)[:, None]
    qq = np.arange(P)[None, :]
    masks = (kk <= qq).astype(NPDT)      # [128, 128]

    in_maps = []
    for core in range(N_CORES):
        b = core // 4
        h0 = (core % 4) * HPC
        cols = np.concatenate(
            [h * DH + perm_head for h in range(h0, h0 + HPC)])   # rope-permuted
        vcols = np.arange(h0 * DH, (h0 + HPC) * DH)              # natural
        in_maps.append({
            "xT": np.ascontiguousarray(x[b].T).astype(NPDT),
            "wq": np.ascontiguousarray(w_q[:, cols]).astype(NPDT),
            "wk": np.ascontiguousarray(w_k[:, cols]).astype(NPDT),
            "wv": np.ascontiguousarray(w_v[:, vcols]).astype(NPDT),
            "wo": np.ascontiguousarray(w_o[vcols, :]).astype(NPDT),
            "cc": cc,
            "ss": ss,
            "masks": masks,
        })
    return in_maps


def kernel(x, w_q, w_k, w_v, w_o, _trace=False, _results_out=None):
    x = np.asarray(x, dtype=np.float32)
    w_q = np.asarray(w_q, dtype=np.float32)
    w_k = np.asarray(w_k, dtype=np.float32)
    w_v = np.asarray(w_v, dtype=np.float32)
    w_o = np.asarray(w_o, dtype=np.float32)
    nc = _build()
    in_maps = _host_inputs(x, w_q, w_k, w_v, w_o)
    res = run_bass_kernel_spmd(
        nc, in_maps, core_ids=list(range(N_CORES)), trace=_trace)
    if _results_out is not None:
        _results_out.append(res)
    out = np.empty((B, S, D), np.float32)
    for b in range(B):
        acc = res.results[4 * b]["resT"].astype(np.float32)
        for g in range(1, 4):
            acc = acc + res.results[4 * b + g]["resT"].astype(np.float32)
        out[b] = acc.T
    return out
